# revision 4
# baseline (speedup 1.0000x reference)
"""DiscriminativeLoss kernel for 8 trn2 NeuronCores (Bass/Tile), v2.

Sharding: core c handles image b = c//2, pixel half h = c%2 (N_s = 524288
pixels per core).  Per core:
  pass 1: per-class counts + segment sums over its pixel shard
          (one-hot matmuls on PE, pixels on the contraction axis),
  AllReduce of per-image [10,18] stats across the 8 cores,
  pass 2: per-pixel hinge-distance sums per class, via 3 accumulating
          matmuls building  s - 2*e.C_k - BIG*(lab-k)^2  on PSUM, then
          relu (DVE, +q[k]-BIG*k^2-dvar^2 bias) and Sqrt (ACT) with
          per-partition accumulation.  Wrong-class lanes land exactly
          at 0 through the whole chain, so the class-masked reduction
          is a plain row sum.

v2 changes vs v1:
  - host converts embedding to bf16 (HBM traffic and host->HBM halved);
  - pass 2 DMAs the same emb tensor in a second (j,e)-major view instead
    of a host-transposed duplicate (embbm gone: 32 MB/core saved);
  - lab and lab^2 are shipped pre-interleaved in pass-2 chunk layout
    (one DMA per chunk, no on-device labsq round-trip through DRAM);
  - engine rebalance in pass 2: relu+accum on DVE, sqrt+accum on ACT,
    e^2 square alternating DVE/GpSimd;
  - pass-2 tiles are prefetched (DMA + square) before the AllReduce so
    the collective latency hides under the prefetch stream.
Host: slices inputs, sums the per-core partial hinge sums and does the
final ~500-flop scalar assembly (centers, pair loss, reg loss, totals).
"""

import os
import sys

import numpy as np

sys.path.insert(0, "/opt/trn_rl_repo")
os.environ.setdefault("MYCRO_LOCAL_CACHE", "1")

import ml_dtypes  # noqa: E402

BF16 = ml_dtypes.bfloat16

# problem constants (hardcoded per harness contract)
B, E, H, W = 4, 16, 1024, 1024
NIMG = H * W
NCORES = 8
NPIX = NIMG // 2            # pixels per core
K = 10
DELTA_VAR = 0.5
DELTA_DST = 1.5
A_W, B_W, R_W = 1.0, 1.0, 0.001
BIG = 1024.0
KJ = 80                      # k-major (8k+j) partition layout size

_cache = {}


def _consts(f1):
    """Host-side constant input arrays shared by all cores."""
    # S2: [128, 80] ones block-diag: S2[16j+e, 8k+j] = 1
    s2 = np.zeros((128, KJ), dtype=np.float32)
    for j in range(8):
        for e in range(E):
            for k in range(K):
                s2[16 * j + e, 8 * k + j] = 1.0
    # S3: [16, 80]: row j = lab-row coeff 2*BIG*k ; row 8+j = lab^2 coeff -BIG
    s3 = np.zeros((16, KJ), dtype=np.float32)
    for j in range(8):
        for k in range(K):
            s3[j, 8 * k + j] = 2.0 * BIG * k
            s3[8 + j, 8 * k + j] = -BIG
    # kpat: [128, 10*f1]: kpat[p, k*f1 + f] = k
    kpat = np.zeros((128, K * f1), dtype=np.float32)
    for k in range(K):
        kpat[:, k * f1:(k + 1) * f1] = float(k)
    # jcol: [80, 10]: jcol[8k+j, k] = 1  (collapse j inside k)
    jcol = np.zeros((KJ, K), dtype=np.float32)
    for k in range(K):
        for j in range(8):
            jcol[8 * k + j, k] = 1.0
    # bk: [80, 1] = -BIG*k^2 - dvar^2
    bk = np.zeros((KJ, 1), dtype=np.float32)
    for k in range(K):
        for j in range(8):
            bk[8 * k + j, 0] = -BIG * k * k - DELTA_VAR * DELTA_VAR
    ones128 = np.ones((128, 1), dtype=np.float32)
    id10 = np.eye(K, dtype=np.float32)
    # qsel: [10, 80]: qsel[k, 8k+j] = 1
    qsel = np.zeros((K, KJ), dtype=np.float32)
    for k in range(K):
        for j in range(8):
            qsel[k, 8 * k + j] = 1.0
    return {
        "qsel": qsel,
        "s2": s2.astype(BF16),
        "s3": s3.astype(BF16),
        "kpat": kpat.astype(BF16),
        "jcol": jcol,
        "bk": bk,
        "ones128": ones128,
        "id10": id10,
    }


def build_module(npix=NPIX, f1=512, f2=512, pref=16):
    """Build the SPMD Bass module (same program on all 8 cores)."""
    import concourse.bass as bass
    import concourse.mybir as mybir
    import concourse.tile as tile
    from concourse import bacc

    f32 = mybir.dt.float32
    bf16 = mybir.dt.bfloat16
    Alu = mybir.AluOpType
    Act = mybir.ActivationFunctionType

    run = npix // 128            # pixel-major run length per partition
    nch1 = run // f1             # pass-1 chunks
    nch2 = npix // (8 * f2)      # pass-2 chunks
    assert run * 128 == npix and nch1 * f1 == run and nch2 * 8 * f2 == npix

    nc = bacc.Bacc(
        "TRN2",
        target_bir_lowering=False,
        debug=False,
        num_devices=NCORES,
    )

    # I/O (emb/labels in bf16 from the host)
    emb_d = nc.dram_tensor("emb", [E, npix], bf16, kind="ExternalInput").ap()
    labf_d = nc.dram_tensor("labf", [npix], bf16, kind="ExternalInput").ap()
    labcat_d = nc.dram_tensor("labcat", [nch2 * 16, f2], bf16,
                              kind="ExternalInput").ap()
    s2_d = nc.dram_tensor("s2", [128, KJ], bf16, kind="ExternalInput").ap()
    s3_d = nc.dram_tensor("s3", [16, KJ], bf16, kind="ExternalInput").ap()
    kpat_d = nc.dram_tensor("kpat", [128, K * f1], bf16, kind="ExternalInput").ap()
    jcol_d = nc.dram_tensor("jcol", [KJ, K], f32, kind="ExternalInput").ap()
    bk_d = nc.dram_tensor("bk", [KJ, 1], f32, kind="ExternalInput").ap()
    bsel_d = nc.dram_tensor("bsel", [K, 4], f32, kind="ExternalInput").ap()
    qsel_d = nc.dram_tensor("qsel", [K, KJ], f32, kind="ExternalInput").ap()
    ones_d = nc.dram_tensor("ones128", [128, 1], f32, kind="ExternalInput").ap()
    id10_d = nc.dram_tensor("id10", [K, K], f32, kind="ExternalInput").ap()

    hpart_d = nc.dram_tensor("hpart", [1, K], f32, kind="ExternalOutput").ap()
    stats_ext = nc.dram_tensor("stats", [K, 4 * 18], f32, kind="ExternalOutput").ap()

    with tile.TileContext(nc) as tc:
        with (
            tc.tile_pool(name="consts", bufs=1) as cp,
            tc.tile_pool(name="p1", bufs=2) as p1,
            tc.tile_pool(name="p2pre", bufs=pref + 2) as p2a,
            tc.tile_pool(name="p2post", bufs=4) as p2b,
            tc.tile_pool(name="ps2", bufs=3, space="PSUM") as psp,
            tc.tile_pool(name="ps1", bufs=1, space="PSUM") as ps1,
            tc.tile_pool(name="dram", bufs=1, space="DRAM") as dp,
        ):
            # ---- persistent constants ----
            s2_t = cp.tile([128, KJ], bf16)
            nc.sync.dma_start(s2_t[:], s2_d[:])
            s3_t = cp.tile([16, KJ], bf16)
            nc.sync.dma_start(s3_t[:], s3_d[:])
            kpat_t = cp.tile([128, K * f1], bf16)
            nc.sync.dma_start(kpat_t[:], kpat_d[:])
            jcol_t = cp.tile([KJ, K], f32)
            nc.sync.dma_start(jcol_t[:], jcol_d[:])
            bk_t = cp.tile([KJ, 1], f32)
            nc.sync.dma_start(bk_t[:], bk_d[:])
            bsel_t = cp.tile([K, 4], f32)
            nc.sync.dma_start(bsel_t[:], bsel_d[:])
            qsel_t = cp.tile([K, KJ], f32)
            nc.sync.dma_start(qsel_t[:], qsel_d[:])
            ones_t = cp.tile([128, 1], f32)
            nc.sync.dma_start(ones_t[:], ones_d[:])
            id10_t = cp.tile([K, K], f32)
            nc.sync.dma_start(id10_t[:], id10_d[:])

            # ---- labels: pixel-major [128, run], bf16 (values 0..9 exact) ----
            lab_pm = cp.tile([128, run], bf16)
            nc.gpsimd.dma_start(lab_pm[:], labf_d.rearrange("(p c) -> p c", p=128))

            # ---- counts: per-class accumulate on pixel-major labels ----
            cnt_acc = cp.tile([128, K], f32)
            cnt_trash = cp.tile([128, run], bf16)
            for k in range(K):
                nc.vector.tensor_scalar(
                    out=cnt_trash[:],
                    in0=lab_pm[:],
                    scalar1=float(k),
                    scalar2=None,
                    op0=Alu.is_equal,
                    op1=Alu.add,
                    accum_out=cnt_acc[:, k:k + 1],
                )
            cnt_ps = ps1.tile([K, 1], f32)
            nc.tensor.matmul(cnt_ps[:], lhsT=cnt_acc[:], rhs=ones_t[:],
                             start=True, stop=True)

            # ---- pass 1: segment sums via per-slab one-hot matmuls ----
            sums_ps = ps1.tile([K, E], f32)
            for c in range(nch1):
                embp = p1.tile([128, E * f1], bf16, tag="embp")
                # dram: emb[e, p*run + c*f1 + f] -> sbuf [p, e*f1 + f]
                src = emb_d.rearrange("e (p c f) -> c p e f", p=128, c=nch1, f=f1)[c]
                nc.gpsimd.dma_start(
                    embp[:].rearrange("p (e f) -> p e f", f=f1), src)
                ohp = p1.tile([128, K * f1], bf16, tag="ohp")
                lab_b = lab_pm[:, c * f1:(c + 1) * f1]
                nc.vector.tensor_tensor(
                    out=ohp[:].rearrange("p (k f) -> p k f", f=f1),
                    in0=lab_b.unsqueeze(1).to_broadcast([128, K, f1]),
                    in1=kpat_t[:].rearrange("p (k f) -> p k f", f=f1),
                    op=Alu.is_equal,
                )
                ohp_v = ohp[:].rearrange("p (k f) -> p f k", f=f1)
                embp_v = embp[:].rearrange("p (e f) -> p f e", f=f1)
                for f in range(f1):
                    nc.tensor.matmul(
                        sums_ps[:],
                        lhsT=ohp_v[:, f, :],
                        rhs=embp_v[:, f, :],
                        start=(c == 0 and f == 0),
                        stop=(c == nch1 - 1 and f == f1 - 1),
                    )

            # ---- pass-2 tile prefetch (DMA + square), pre-collective ----
            emb2_r = emb_d.rearrange("e (c j f) -> c (j e) f", c=nch2, j=8, f=f2)
            labcat_r = labcat_d.rearrange("(c r) f -> c r f", r=16)

            emb_tiles = {}
            e2_tiles = {}
            lp_tiles = {}

            def fetch_chunk(c):
                embB = p2a.tile([128, f2], bf16, tag="embB")
                nc.gpsimd.dma_start(embB[:], emb2_r[c])
                lp = p2a.tile([16, f2], bf16, tag="lp")
                nc.gpsimd.dma_start(lp[:], labcat_r[c])
                e2B = p2a.tile([128, f2], bf16, tag="e2B")
                eng = nc.vector if (c % 2 == 0) else nc.gpsimd
                eng.tensor_tensor(e2B[:], embB[:], embB[:], op=Alu.mult)
                emb_tiles[c] = embB
                lp_tiles[c] = lp
                e2_tiles[c] = e2B

            for c in range(min(pref, nch2)):
                fetch_chunk(c)

            # ---- stats block [10, 18]: col0 counts, col1..16 sums ----
            stats_blk = cp.tile([K, 18], f32)
            nc.vector.memset(stats_blk[:], 0.0)
            nc.scalar.copy(stats_blk[:, 0:1], cnt_ps[:])
            nc.scalar.copy(stats_blk[:, 1:1 + E], sums_ps[:])

            # scatter to [10, 4*18] weighted by per-core bsel (one-hot on b)
            stats40 = cp.tile([K, 4 * 18], f32)
            for b in range(4):
                nc.vector.tensor_scalar(
                    out=stats40[:, 18 * b:18 * (b + 1)],
                    in0=stats_blk[:],
                    scalar1=bsel_t[:, b:b + 1],
                    scalar2=None,
                    op0=Alu.mult,
                )

            # ---- AllReduce stats across the 8 cores ----
            cc_in = dp.tile([K, 4 * 18], f32)
            cc_out = dp.tile([K, 4 * 18], f32, addr_space="Shared")
            nc.sync.dma_start(cc_in[:], stats40[:])
            nc.gpsimd.collective_compute(
                "AllReduce",
                mybir.AluOpType.add,
                replica_groups=[list(range(NCORES))],
                ins=[cc_in[:].opt()],
                outs=[cc_out[:].opt()],
            )
            stats_all = cp.tile([K, 4 * 18], f32)
            nc.sync.dma_start(stats_all[:], cc_out[:])
            nc.sync.dma_start(stats_ext[:], cc_out[:])

            # ---- own-image stats: myst = sum_b bsel[b]*stats_all[b] ----
            mya = cp.tile([K, 18], f32)
            myb = cp.tile([K, 18], f32)
            nc.vector.tensor_scalar(
                out=mya[:], in0=stats_all[:, 0:18],
                scalar1=bsel_t[:, 0:1], scalar2=None, op0=Alu.mult)
            srcs = [mya, myb]
            for b in range(1, 4):
                si, so = srcs[(b - 1) % 2], srcs[b % 2]
                nc.vector.scalar_tensor_tensor(
                    out=so[:],
                    in0=stats_all[:, 18 * b:18 * (b + 1)],
                    scalar=bsel_t[:, b:b + 1],
                    in1=si[:],
                    op0=Alu.mult,
                    op1=Alu.add,
                )
            myst = srcs[3 % 2]  # = myb

            # ---- centers, q, stationaries for pass 2 ----
            cnt_safe = cp.tile([K, 1], f32)
            nc.vector.tensor_scalar(out=cnt_safe[:], in0=myst[:, 0:1],
                                    scalar1=1.0, scalar2=None, op0=Alu.max)
            rec = cp.tile([K, 1], f32)
            nc.vector.reciprocal(rec[:], cnt_safe[:])
            cmat = cp.tile([K, E], f32)
            nc.vector.tensor_scalar(out=cmat[:], in0=myst[:, 1:1 + E],
                                    scalar1=rec[:, 0:1], scalar2=None,
                                    op0=Alu.mult)
            csq = cp.tile([K, E], f32)
            nc.vector.tensor_tensor(csq[:], cmat[:], cmat[:], op=Alu.mult)
            qv = cp.tile([K, 1], f32)
            nc.vector.tensor_reduce(qv[:], csq[:], mybir.AxisListType.X, Alu.add)

            ct_ps = ps1.tile([E, K], f32)
            nc.tensor.matmul(ct_ps[:], lhsT=cmat[:], rhs=id10_t[:],
                             start=True, stop=True)
            ctb = cp.tile([E, K], bf16)
            nc.scalar.copy(ctb[:], ct_ps[:])

            ctbm = cp.tile([E, K], bf16)
            nc.vector.tensor_scalar(out=ctbm[:], in0=ctb[:], scalar1=-2.0,
                                    scalar2=None, op0=Alu.mult)
            s1_t = cp.tile([128, KJ], bf16)
            nc.vector.memset(s1_t[:], 0.0)
            s1_v = s1_t[:].rearrange("p (k j) -> p j k", j=8)
            for j in range(8):
                nc.gpsimd.dma_start(
                    s1_v[16 * j:16 * (j + 1), j, :], ctbm[:])
            qb_ps = ps1.tile([KJ, 1], f32)
            nc.tensor.matmul(qb_ps[:], lhsT=qsel_t[:], rhs=qv[:],
                             start=True, stop=True)
            qb2 = cp.tile([KJ, 1], f32)
            nc.scalar.activation(qb2[:], qb_ps[:], Act.Identity,
                                 bias=bk_t[:, 0:1], scale=1.0)
            dv2 = cp.tile([KJ, 1], f32)
            nc.vector.memset(dv2[:], DELTA_VAR * DELTA_VAR)

            # ---- pass 2 ----
            uacc = cp.tile([KJ, nch2], f32)
            yacc = cp.tile([KJ, nch2], f32)
            for c in range(nch2):
                if c + pref < nch2:
                    fetch_chunk(c + pref)
                embB = emb_tiles.pop(c)
                e2B = e2_tiles.pop(c)
                lp = lp_tiles.pop(c)

                ps2t = psp.tile([KJ, f2], f32, tag="ps2")
                nc.tensor.matmul(ps2t[:], lhsT=s1_t[:], rhs=embB[:],
                                 start=True, stop=False)
                nc.tensor.matmul(ps2t[:], lhsT=s2_t[:], rhs=e2B[:],
                                 start=False, stop=False)
                nc.tensor.matmul(ps2t[:], lhsT=s3_t[:], rhs=lp[:],
                                 start=False, stop=True)

                # u = relu(ps2 + qb2), row-accumulated, on DVE
                u_t = p2b.tile([KJ, f2], bf16, tag="u")
                nc.vector.tensor_scalar(
                    out=u_t[:],
                    in0=ps2t[:],
                    scalar1=qb2[:, 0:1],
                    scalar2=0.0,
                    op0=Alu.add,
                    op1=Alu.max,
                    accum_out=uacc[:, c:c + 1],
                )
                # y = sqrt(u + dvar^2), row-accumulated, on ACT
                tr_t = p2b.tile([KJ, f2], bf16, tag="tr")
                nc.scalar.activation(tr_t[:], u_t[:], Act.Sqrt,
                                     bias=dv2[:, 0:1], scale=1.0,
                                     accum_out=yacc[:, c:c + 1])

            # ---- H assembly: H_p = sum(u) - 2*dvar*sum(y) + 2*dvar^2*Npp ----
            u1 = cp.tile([KJ, 1], f32)
            y1 = cp.tile([KJ, 1], f32)
            nc.vector.tensor_reduce(u1[:], uacc[:], mybir.AxisListType.X, Alu.add)
            nc.vector.tensor_reduce(y1[:], yacc[:], mybir.AxisListType.X, Alu.add)
            hp = cp.tile([KJ, 1], f32)
            nc.vector.scalar_tensor_tensor(
                out=hp[:], in0=y1[:], scalar=-2.0 * DELTA_VAR, in1=u1[:],
                op0=Alu.mult, op1=Alu.add)
            npp = float(f2 * nch2)
            hp2 = cp.tile([KJ, 1], f32)
            nc.vector.tensor_scalar(
                out=hp2[:], in0=hp[:],
                scalar1=2.0 * DELTA_VAR * DELTA_VAR * npp,
                scalar2=None, op0=Alu.add)
            h_ps = ps1.tile([1, K], f32)
            nc.tensor.matmul(h_ps[:], lhsT=hp2[:], rhs=jcol_t[:],
                             start=True, stop=True)
            h_sb = cp.tile([1, K], f32)
            nc.scalar.copy(h_sb[:], h_ps[:])
            nc.sync.dma_start(hpart_d[:], h_sb[:])

    nc.compile()
    return nc


def _host_finalize(stats, hsum):
    """stats: [4, 10, 18] float64-ready; hsum: [4, 10] summed hinge partials."""
    lv_l, ld_l, lr_l, valid_l = [], [], [], []
    ids = np.arange(K)
    for b in range(B):
        counts = stats[b, :, 0].astype(np.float64)
        sums = stats[b, :, 1:1 + E].astype(np.float64)
        present = (counts > 0) & (ids > 0)
        presf = present.astype(np.float64)
        safe = np.where(counts > 0, counts, 1.0)
        centers = sums / safe[:, None]
        per_inst = hsum[b].astype(np.float64) / safe
        n_inst = presf.sum()
        lv = float((per_inst * presf).sum() / max(n_inst, 1.0))
        cdiff = centers[:, None, :] - centers[None, :, :]
        csq = (cdiff * cdiff).sum(-1)
        pm = present[:, None] & present[None, :] & (ids[:, None] < ids[None, :])
        cdist = np.sqrt(np.where(pm, csq, 1.0))
        ph = np.square(np.maximum(2.0 * DELTA_DST - cdist, 0.0)) * pm
        n_pairs = pm.sum()
        ld = float(ph.sum() / max(n_pairs, 1.0))
        cn = np.sqrt(np.where(present, (centers * centers).sum(-1), 1.0))
        lr = float((cn * presf).sum() / max(n_inst, 1.0))
        valid = 1.0 if n_inst > 0 else 0.0
        lv_l.append(lv * valid)
        ld_l.append(ld * valid)
        lr_l.append(lr * valid)
        valid_l.append(valid)
    vb = max(sum(valid_l), 1.0)
    loss_var = sum(lv_l) / vb
    loss_dst = sum(ld_l) / vb
    loss_reg = sum(lr_l) / vb
    total = A_W * loss_var + B_W * loss_dst + R_W * loss_reg
    return (
        np.float32(total),
        np.float32(loss_var),
        np.float32(loss_dst),
        np.float32(loss_reg),
    )


LAST_RES = None

F2 = 512
NCH2 = NPIX // (8 * F2)


def kernel(embedding, ins_label):
    global LAST_RES
    from concourse.bass_utils import run_bass_kernel_spmd

    key = "mod"
    if key not in _cache:
        _cache[key] = build_module()
    nc = _cache[key]

    consts = _consts(512)
    emb_r = np.asarray(embedding, dtype=np.float32).reshape(B, E, NIMG)
    lab_r = np.asarray(ins_label).reshape(B, NIMG).astype(np.float32)

    in_maps = []
    for c in range(NCORES):
        b, h = c // 2, c % 2
        sl = slice(h * NPIX, (h + 1) * NPIX)
        bsel = np.zeros((K, 4), dtype=np.float32)
        bsel[:, b] = 1.0
        m = dict(consts)
        m["emb"] = np.ascontiguousarray(emb_r[b, :, sl]).astype(BF16)
        labn = np.ascontiguousarray(lab_r[b, sl])
        m["labf"] = labn.astype(BF16)
        # labcat: per pass-2 chunk, rows 0..7 = lab(j), rows 8..15 = lab^2(j)
        labv = labn.reshape(NCH2, 8, F2)
        labcat = np.concatenate([labv, labv * labv], axis=1)  # [NCH2, 16, F2]
        m["labcat"] = labcat.reshape(NCH2 * 16, F2).astype(BF16)
        m["bsel"] = bsel
        in_maps.append(m)

    trace = os.environ.get("KTRACE", "") == "1"
    kw = {}
    if trace:
        kw["trace"] = True
        td = os.environ.get("KTRACE_DIR")
        if td:
            os.makedirs(td, exist_ok=True)
            kw["tmpdir"] = td
    res = run_bass_kernel_spmd(nc, in_maps, core_ids=list(range(NCORES)), **kw)
    LAST_RES = res
    stats = (res.results[0]["stats"].astype(np.float64)
             .reshape(K, 4, 18).transpose(1, 0, 2))
    hsum = np.zeros((B, K), dtype=np.float64)
    for c in range(NCORES):
        hsum[c // 2] += res.results[c]["hpart"].astype(np.float64).reshape(K)
    return _host_finalize(stats, hsum)


if __name__ == "__main__":
    # smoke build
    build_module()
    print("build ok")


# revision 6
# speedup vs baseline: 1.2606x; 1.2606x over previous
"""DiscriminativeLoss kernel for 8 trn2 NeuronCores (Bass/Tile), v2.

Sharding: core c handles image b = c//2, pixel half h = c%2 (N_s = 524288
pixels per core).  Per core:
  pass 1: per-class segment sums over its pixel shard (one-hot matmuls on
          PE, pixels on the contraction axis); per-class counts ride in
          from the host (trivial bincount) and join the stats block,
  AllReduce of per-image [10,18] stats across the 8 cores,
  pass 2: per-pixel hinge-distance sums per class, via 3 accumulating
          matmuls building  s - 2*e.C_k - BIG*(lab-k)^2  on PSUM, then
          relu (DVE, +q[k]-BIG*k^2-dvar^2 bias, row-accumulated) and a
          4-chunk-batched Sqrt (ACT, row-accumulated).  Wrong-class lanes
          land exactly at 0 through the whole chain, so the class-masked
          reduction is a plain row sum.

v2 vs v1:
  - emb ships as bf16 (pass 1) + fp8 e4m3 (pass 2) + fp8 e^2 (pass 2),
    labels as bf16 with lab/lab^2 pre-interleaved per pass-2 super-chunk:
    HBM traffic ~35 MB/core vs 70 MB, and the on-device e^2 square is gone;
  - pass-2 DMAs are super-chunked (4 compute chunks per DMA) to amortize
    the ~640 ns per-dma_start issuance cost on the Pool queue;
  - pass-2 engine split: relu+accum on DVE, sqrt+accum on ACT batched over
    4 chunks (amortizes ACT fixed overheads + accumulator reads);
  - pass-2 tiles prefetch before/during the AllReduce so the collective
    hides under the DMA stream;
  - host-computed counts remove the 10-op DVE counts loop from startup.
Host: slices/converts inputs, sums the per-core partial hinge sums and
does the final ~500-flop scalar assembly.
"""

import os
import sys

import numpy as np

sys.path.insert(0, "/opt/trn_rl_repo")
os.environ.setdefault("MYCRO_LOCAL_CACHE", "1")

import ml_dtypes  # noqa: E402

BF16 = ml_dtypes.bfloat16
FP8 = ml_dtypes.float8_e4m3

# problem constants (hardcoded per harness contract)
B, E, H, W = 4, 16, 1024, 1024
NIMG = H * W
NCORES = 8
NPIX = NIMG // 2            # pixels per core
K = 10
DELTA_VAR = 0.5
DELTA_DST = 1.5
A_W, B_W, R_W = 1.0, 1.0, 0.001
BIG = 1024.0
KJ = 80                      # k-major (8k+j) partition layout size
F1 = 512                     # pass-1 chunk columns
F2 = 512                     # pass-2 compute chunk columns (PSUM bank)
SC = 4                       # pass-2 chunks per DMA super-chunk
FS = F2 * SC                 # super-chunk columns
NCH2 = NPIX // (8 * F2)      # pass-2 compute chunks
NSUP = NCH2 // SC            # pass-2 super chunks
PREF_S = 6                   # supers prefetched ahead

_cache = {}


def _consts(f1):
    """Host-side constant input arrays shared by all cores."""
    # S2: [128, 80] ones block-diag: S2[16j+e, 8k+j] = 1
    s2 = np.zeros((128, KJ), dtype=np.float32)
    for j in range(8):
        for e in range(E):
            for k in range(K):
                s2[16 * j + e, 8 * k + j] = 1.0
    # S3: [16, 80]: row j = lab-row coeff 2*BIG*k ; row 8+j = lab^2 coeff -BIG
    s3 = np.zeros((16, KJ), dtype=np.float32)
    for j in range(8):
        for k in range(K):
            s3[j, 8 * k + j] = 2.0 * BIG * k
            s3[8 + j, 8 * k + j] = -BIG
    # kpat: [128, 10*f1]: kpat[p, k*f1 + f] = k
    kpat = np.zeros((128, K * f1), dtype=np.float32)
    for k in range(K):
        kpat[:, k * f1:(k + 1) * f1] = float(k)
    # jcol: [80, 10]: jcol[8k+j, k] = 1  (collapse j inside k)
    jcol = np.zeros((KJ, K), dtype=np.float32)
    for k in range(K):
        for j in range(8):
            jcol[8 * k + j, k] = 1.0
    # bk: [80, 1] = -BIG*k^2 - dvar^2
    bk = np.zeros((KJ, 1), dtype=np.float32)
    for k in range(K):
        for j in range(8):
            bk[8 * k + j, 0] = -BIG * k * k - DELTA_VAR * DELTA_VAR
    id10 = np.eye(K, dtype=np.float32)
    # qsel: [10, 80]: qsel[k, 8k+j] = 1
    qsel = np.zeros((K, KJ), dtype=np.float32)
    for k in range(K):
        for j in range(8):
            qsel[k, 8 * k + j] = 1.0
    return {
        "qsel": qsel,
        "s2": s2.astype(BF16),
        "s3": s3.astype(BF16),
        "kpat": kpat.astype(BF16),
        "jcol": jcol,
        "bk": bk,
        "id10": id10,
    }


def build_module(npix=NPIX, f1=F1, f2=F2, sc=SC, pref_s=PREF_S):
    """Build the SPMD Bass module (same program on all 8 cores)."""
    import concourse.bass as bass
    import concourse.mybir as mybir
    import concourse.tile as tile
    from concourse import bacc

    f32 = mybir.dt.float32
    bf16 = mybir.dt.bfloat16
    fp8 = mybir.dt.float8e4
    Alu = mybir.AluOpType
    Act = mybir.ActivationFunctionType

    run = npix // 128            # pixel-major run length per partition
    nch1 = run // f1             # pass-1 chunks
    nch2 = npix // (8 * f2)      # pass-2 chunks
    fs = f2 * sc
    nsup = nch2 // sc
    assert run * 128 == npix and nch1 * f1 == run and nsup * sc == nch2

    nc = bacc.Bacc(
        "TRN2",
        target_bir_lowering=False,
        debug=False,
        num_devices=NCORES,
    )

    # I/O
    emb_d = nc.dram_tensor("emb", [E, npix], bf16, kind="ExternalInput").ap()
    emb8_d = nc.dram_tensor("emb8", [E, npix], fp8, kind="ExternalInput").ap()
    esq8_d = nc.dram_tensor("esq8", [E, npix], fp8, kind="ExternalInput").ap()
    labf_d = nc.dram_tensor("labf", [npix], bf16, kind="ExternalInput").ap()
    labcat_d = nc.dram_tensor("labcat", [nsup * 16, fs], bf16,
                              kind="ExternalInput").ap()
    cnt_d = nc.dram_tensor("cnt", [K, 1], f32, kind="ExternalInput").ap()
    s2_d = nc.dram_tensor("s2", [128, KJ], bf16, kind="ExternalInput").ap()
    s3_d = nc.dram_tensor("s3", [16, KJ], bf16, kind="ExternalInput").ap()
    kpat_d = nc.dram_tensor("kpat", [128, K * f1], bf16, kind="ExternalInput").ap()
    jcol_d = nc.dram_tensor("jcol", [KJ, K], f32, kind="ExternalInput").ap()
    bk_d = nc.dram_tensor("bk", [KJ, 1], f32, kind="ExternalInput").ap()
    bsel_d = nc.dram_tensor("bsel", [K, 4], f32, kind="ExternalInput").ap()
    qsel_d = nc.dram_tensor("qsel", [K, KJ], f32, kind="ExternalInput").ap()
    id10_d = nc.dram_tensor("id10", [K, K], f32, kind="ExternalInput").ap()

    hpart_d = nc.dram_tensor("hpart", [1, K], f32, kind="ExternalOutput").ap()
    stats_ext = nc.dram_tensor("stats", [K, 4 * 18], f32, kind="ExternalOutput").ap()

    with tile.TileContext(nc) as tc:
        with (
            tc.tile_pool(name="consts", bufs=1) as cp,
            tc.tile_pool(name="p1", bufs=2) as p1,
            tc.tile_pool(name="p2pre", bufs=pref_s + 2) as p2a,
            tc.tile_pool(name="p2post", bufs=3) as p2b,
            tc.tile_pool(name="ps2", bufs=3, space="PSUM") as psp,
            tc.tile_pool(name="ps1", bufs=1, space="PSUM") as ps1,
            tc.tile_pool(name="dram", bufs=1, space="DRAM") as dp,
        ):
            # ---- persistent constants ----
            s2_t = cp.tile([128, KJ], bf16)
            nc.sync.dma_start(s2_t[:], s2_d[:])
            s3_t = cp.tile([16, KJ], bf16)
            nc.sync.dma_start(s3_t[:], s3_d[:])
            kpat_t = cp.tile([128, K * f1], bf16)
            nc.sync.dma_start(kpat_t[:], kpat_d[:])
            jcol_t = cp.tile([KJ, K], f32)
            nc.sync.dma_start(jcol_t[:], jcol_d[:])
            bk_t = cp.tile([KJ, 1], f32)
            nc.sync.dma_start(bk_t[:], bk_d[:])
            bsel_t = cp.tile([K, 4], f32)
            nc.sync.dma_start(bsel_t[:], bsel_d[:])
            qsel_t = cp.tile([K, KJ], f32)
            nc.sync.dma_start(qsel_t[:], qsel_d[:])
            id10_t = cp.tile([K, K], f32)
            nc.sync.dma_start(id10_t[:], id10_d[:])
            cnt_t = cp.tile([K, 1], f32)
            nc.sync.dma_start(cnt_t[:], cnt_d[:])

            # ---- labels: pixel-major [128, run], bf16 (values 0..9 exact) ----
            lab_pm = cp.tile([128, run], bf16)
            nc.gpsimd.dma_start(lab_pm[:], labf_d.rearrange("(p c) -> p c", p=128))

            # ---- pass 1: segment sums via per-slab one-hot matmuls ----
            sums_ps = ps1.tile([K, E], f32)
            for c in range(nch1):
                embp = p1.tile([128, E * f1], bf16, tag="embp")
                # dram: emb[e, p*run + c*f1 + f] -> sbuf [p, e*f1 + f]
                src = emb_d.rearrange("e (p c f) -> c p e f", p=128, c=nch1, f=f1)[c]
                nc.gpsimd.dma_start(
                    embp[:].rearrange("p (e f) -> p e f", f=f1), src)
                ohp = p1.tile([128, K * f1], bf16, tag="ohp")
                lab_b = lab_pm[:, c * f1:(c + 1) * f1]
                nc.vector.tensor_tensor(
                    out=ohp[:].rearrange("p (k f) -> p k f", f=f1),
                    in0=lab_b.unsqueeze(1).to_broadcast([128, K, f1]),
                    in1=kpat_t[:].rearrange("p (k f) -> p k f", f=f1),
                    op=Alu.is_equal,
                )
                ohp_v = ohp[:].rearrange("p (k f) -> p f k", f=f1)
                embp_v = embp[:].rearrange("p (e f) -> p f e", f=f1)
                for f in range(f1):
                    nc.tensor.matmul(
                        sums_ps[:],
                        lhsT=ohp_v[:, f, :],
                        rhs=embp_v[:, f, :],
                        start=(c == 0 and f == 0),
                        stop=(c == nch1 - 1 and f == f1 - 1),
                    )

            # ---- pass-2 super-chunk prefetch (DMA only), pre-collective ----
            emb2_r = emb8_d.rearrange("e (s j f) -> s j e f", s=nsup, j=8, f=fs)
            esq2_r = esq8_d.rearrange("e (s j f) -> s j e f", s=nsup, j=8, f=fs)
            labcat_r = labcat_d.rearrange("(s r) f -> s r f", r=16)

            es_tiles = {}
            qs_tiles = {}
            ls_tiles = {}

            def fetch_super(s):
                es = p2a.tile([128, fs], fp8, tag="es")
                nc.gpsimd.dma_start(
                    es[:].rearrange("(j e) f -> j e f", j=8), emb2_r[s])
                qs = p2a.tile([128, fs], fp8, tag="qs")
                nc.gpsimd.dma_start(
                    qs[:].rearrange("(j e) f -> j e f", j=8), esq2_r[s])
                ls = p2a.tile([16, fs], bf16, tag="ls")
                nc.gpsimd.dma_start(ls[:], labcat_r[s])
                es_tiles[s] = es
                qs_tiles[s] = qs
                ls_tiles[s] = ls

            for s in range(min(pref_s, nsup)):
                fetch_super(s)

            # ---- stats block [10, 18]: col0 counts, col1..16 sums ----
            stats_blk = cp.tile([K, 18], f32)
            nc.vector.memset(stats_blk[:], 0.0)
            nc.scalar.copy(stats_blk[:, 0:1], cnt_t[:])
            nc.scalar.copy(stats_blk[:, 1:1 + E], sums_ps[:])

            # scatter to [10, 4*18] weighted by per-core bsel (one-hot on b)
            stats40 = cp.tile([K, 4 * 18], f32)
            for b in range(4):
                nc.vector.tensor_scalar(
                    out=stats40[:, 18 * b:18 * (b + 1)],
                    in0=stats_blk[:],
                    scalar1=bsel_t[:, b:b + 1],
                    scalar2=None,
                    op0=Alu.mult,
                )

            # ---- AllReduce stats across the 8 cores ----
            cc_in = dp.tile([K, 4 * 18], f32)
            cc_out = dp.tile([K, 4 * 18], f32, addr_space="Shared")
            nc.sync.dma_start(cc_in[:], stats40[:])
            nc.gpsimd.collective_compute(
                "AllReduce",
                mybir.AluOpType.add,
                replica_groups=[list(range(NCORES))],
                ins=[cc_in[:].opt()],
                outs=[cc_out[:].opt()],
            )
            stats_all = cp.tile([K, 4 * 18], f32)
            nc.sync.dma_start(stats_all[:], cc_out[:])
            nc.sync.dma_start(stats_ext[:], cc_out[:])

            # ---- own-image stats: myst = sum_b bsel[b]*stats_all[b] ----
            mya = cp.tile([K, 18], f32)
            myb = cp.tile([K, 18], f32)
            nc.vector.tensor_scalar(
                out=mya[:], in0=stats_all[:, 0:18],
                scalar1=bsel_t[:, 0:1], scalar2=None, op0=Alu.mult)
            srcs = [mya, myb]
            for b in range(1, 4):
                si, so = srcs[(b - 1) % 2], srcs[b % 2]
                nc.vector.scalar_tensor_tensor(
                    out=so[:],
                    in0=stats_all[:, 18 * b:18 * (b + 1)],
                    scalar=bsel_t[:, b:b + 1],
                    in1=si[:],
                    op0=Alu.mult,
                    op1=Alu.add,
                )
            myst = srcs[3 % 2]  # = myb

            # ---- centers, q, stationaries for pass 2 ----
            cnt_safe = cp.tile([K, 1], f32)
            nc.vector.tensor_scalar(out=cnt_safe[:], in0=myst[:, 0:1],
                                    scalar1=1.0, scalar2=None, op0=Alu.max)
            rec = cp.tile([K, 1], f32)
            nc.vector.reciprocal(rec[:], cnt_safe[:])
            cmat = cp.tile([K, E], f32)
            nc.vector.tensor_scalar(out=cmat[:], in0=myst[:, 1:1 + E],
                                    scalar1=rec[:, 0:1], scalar2=None,
                                    op0=Alu.mult)
            csq = cp.tile([K, E], f32)
            nc.vector.tensor_tensor(csq[:], cmat[:], cmat[:], op=Alu.mult)
            qv = cp.tile([K, 1], f32)
            nc.vector.tensor_reduce(qv[:], csq[:], mybir.AxisListType.X, Alu.add)

            ct_ps = ps1.tile([E, K], f32)
            nc.tensor.matmul(ct_ps[:], lhsT=cmat[:], rhs=id10_t[:],
                             start=True, stop=True)
            ctb = cp.tile([E, K], bf16)
            nc.scalar.copy(ctb[:], ct_ps[:])

            ctbm = cp.tile([E, K], bf16)
            nc.vector.tensor_scalar(out=ctbm[:], in0=ctb[:], scalar1=-2.0,
                                    scalar2=None, op0=Alu.mult)
            s1_t = cp.tile([128, KJ], bf16)
            nc.vector.memset(s1_t[:], 0.0)
            s1_v = s1_t[:].rearrange("p (k j) -> p j k", j=8)
            for j in range(8):
                nc.gpsimd.dma_start(
                    s1_v[16 * j:16 * (j + 1), j, :], ctbm[:])
            qb_ps = ps1.tile([KJ, 1], f32)
            nc.tensor.matmul(qb_ps[:], lhsT=qsel_t[:], rhs=qv[:],
                             start=True, stop=True)
            qb2 = cp.tile([KJ, 1], f32)
            nc.scalar.activation(qb2[:], qb_ps[:], Act.Identity,
                                 bias=bk_t[:, 0:1], scale=1.0)
            dv2 = cp.tile([KJ, 1], f32)
            nc.vector.memset(dv2[:], DELTA_VAR * DELTA_VAR)

            # ---- pass 2 ----
            uacc = cp.tile([KJ, nch2], f32)
            yacc = cp.tile([KJ, nsup], f32)
            for s in range(nsup):
                if s + pref_s < nsup:
                    fetch_super(s + pref_s)
                es = es_tiles.pop(s)
                qs = qs_tiles.pop(s)
                ls = ls_tiles.pop(s)

                u_t = p2b.tile([KJ, fs], bf16, tag="u")
                for t in range(sc):
                    c = s * sc + t
                    sl = slice(t * f2, (t + 1) * f2)
                    ps2t = psp.tile([KJ, f2], f32, tag="ps2")
                    nc.tensor.matmul(ps2t[:], lhsT=s1_t[:], rhs=es[:, sl],
                                     start=True, stop=False)
                    nc.tensor.matmul(ps2t[:], lhsT=s2_t[:], rhs=qs[:, sl],
                                     start=False, stop=False)
                    nc.tensor.matmul(ps2t[:], lhsT=s3_t[:], rhs=ls[:, sl],
                                     start=False, stop=True)
                    # u = relu(ps2 + qb2), row-accumulated, on DVE
                    nc.vector.tensor_scalar(
                        out=u_t[:, sl],
                        in0=ps2t[:],
                        scalar1=qb2[:, 0:1],
                        scalar2=0.0,
                        op0=Alu.add,
                        op1=Alu.max,
                        accum_out=uacc[:, c:c + 1],
                    )
                # y = sqrt(u + dvar^2), batched over the super, on ACT
                tr_t = p2b.tile([KJ, fs], bf16, tag="tr")
                nc.scalar.activation(tr_t[:], u_t[:], Act.Sqrt,
                                     bias=dv2[:, 0:1], scale=1.0,
                                     accum_out=yacc[:, s:s + 1])

            # ---- H assembly: H_p = sum(u) - 2*dvar*sum(y) + 2*dvar^2*Npp ----
            u1 = cp.tile([KJ, 1], f32)
            y1 = cp.tile([KJ, 1], f32)
            nc.vector.tensor_reduce(u1[:], uacc[:], mybir.AxisListType.X, Alu.add)
            nc.vector.tensor_reduce(y1[:], yacc[:], mybir.AxisListType.X, Alu.add)
            hp = cp.tile([KJ, 1], f32)
            nc.vector.scalar_tensor_tensor(
                out=hp[:], in0=y1[:], scalar=-2.0 * DELTA_VAR, in1=u1[:],
                op0=Alu.mult, op1=Alu.add)
            npp = float(f2 * nch2)
            hp2 = cp.tile([KJ, 1], f32)
            nc.vector.tensor_scalar(
                out=hp2[:], in0=hp[:],
                scalar1=2.0 * DELTA_VAR * DELTA_VAR * npp,
                scalar2=None, op0=Alu.add)
            h_ps = ps1.tile([1, K], f32)
            nc.tensor.matmul(h_ps[:], lhsT=hp2[:], rhs=jcol_t[:],
                             start=True, stop=True)
            h_sb = cp.tile([1, K], f32)
            nc.scalar.copy(h_sb[:], h_ps[:])
            nc.sync.dma_start(hpart_d[:], h_sb[:])

    nc.compile()
    return nc


def _host_finalize(stats, hsum):
    """stats: [4, 10, 18] float64-ready; hsum: [4, 10] summed hinge partials."""
    lv_l, ld_l, lr_l, valid_l = [], [], [], []
    ids = np.arange(K)
    for b in range(B):
        counts = stats[b, :, 0].astype(np.float64)
        sums = stats[b, :, 1:1 + E].astype(np.float64)
        present = (counts > 0) & (ids > 0)
        presf = present.astype(np.float64)
        safe = np.where(counts > 0, counts, 1.0)
        centers = sums / safe[:, None]
        per_inst = hsum[b].astype(np.float64) / safe
        n_inst = presf.sum()
        lv = float((per_inst * presf).sum() / max(n_inst, 1.0))
        cdiff = centers[:, None, :] - centers[None, :, :]
        csq = (cdiff * cdiff).sum(-1)
        pm = present[:, None] & present[None, :] & (ids[:, None] < ids[None, :])
        cdist = np.sqrt(np.where(pm, csq, 1.0))
        ph = np.square(np.maximum(2.0 * DELTA_DST - cdist, 0.0)) * pm
        n_pairs = pm.sum()
        ld = float(ph.sum() / max(n_pairs, 1.0))
        cn = np.sqrt(np.where(present, (centers * centers).sum(-1), 1.0))
        lr = float((cn * presf).sum() / max(n_inst, 1.0))
        valid = 1.0 if n_inst > 0 else 0.0
        lv_l.append(lv * valid)
        ld_l.append(ld * valid)
        lr_l.append(lr * valid)
        valid_l.append(valid)
    vb = max(sum(valid_l), 1.0)
    loss_var = sum(lv_l) / vb
    loss_dst = sum(ld_l) / vb
    loss_reg = sum(lr_l) / vb
    total = A_W * loss_var + B_W * loss_dst + R_W * loss_reg
    return (
        np.float32(total),
        np.float32(loss_var),
        np.float32(loss_dst),
        np.float32(loss_reg),
    )


LAST_RES = None


def kernel(embedding, ins_label):
    global LAST_RES
    from concourse.bass_utils import run_bass_kernel_spmd

    key = "mod"
    if key not in _cache:
        _cache[key] = build_module()
    nc = _cache[key]

    consts = _consts(F1)
    emb_r = np.asarray(embedding, dtype=np.float32).reshape(B, E, NIMG)
    lab_r = np.asarray(ins_label).reshape(B, NIMG)

    in_maps = []
    for c in range(NCORES):
        b, h = c // 2, c % 2
        sl = slice(h * NPIX, (h + 1) * NPIX)
        bsel = np.zeros((K, 4), dtype=np.float32)
        bsel[:, b] = 1.0
        m = dict(consts)
        esh = np.ascontiguousarray(emb_r[b, :, sl])
        m["emb"] = esh.astype(BF16)
        m["emb8"] = esh.astype(FP8)
        m["esq8"] = (esh * esh).astype(FP8)
        labn = lab_r[b, sl].astype(np.float32)
        m["labf"] = labn.astype(BF16)
        # labcat: per pass-2 super-chunk, rows 0..7 = lab(j), 8..15 = lab^2(j)
        labv = labn.reshape(NSUP, 8, FS)
        labcat = np.concatenate([labv, labv * labv], axis=1)  # [NSUP, 16, FS]
        m["labcat"] = labcat.reshape(NSUP * 16, FS).astype(BF16)
        m["cnt"] = np.bincount(
            lab_r[b, sl].astype(np.int64), minlength=K
        )[:K].astype(np.float32).reshape(K, 1)
        m["bsel"] = bsel
        in_maps.append(m)

    trace = os.environ.get("KTRACE", "") == "1"
    kw = {}
    if trace:
        kw["trace"] = True
        td = os.environ.get("KTRACE_DIR")
        if td:
            os.makedirs(td, exist_ok=True)
            kw["tmpdir"] = td
    res = run_bass_kernel_spmd(nc, in_maps, core_ids=list(range(NCORES)), **kw)
    LAST_RES = res
    stats = (res.results[0]["stats"].astype(np.float64)
             .reshape(K, 4, 18).transpose(1, 0, 2))
    hsum = np.zeros((B, K), dtype=np.float64)
    for c in range(NCORES):
        hsum[c // 2] += res.results[c]["hpart"].astype(np.float64).reshape(K)
    return _host_finalize(stats, hsum)


if __name__ == "__main__":
    # smoke build
    build_module()
    print("build ok")
